# revision 48
# baseline (speedup 1.0000x reference)
"""Trainium2 Bass kernel for nn_Angles2Backbone.

Full inputs:  input [1024, 3, 512] f32 (phi/psi/omega dihedrals), angles_length [1024] i64.
Full output:  [1024, 4608] f32 backbone coords (N, CA, C per residue, xyz interleaved).

Strategy: pure data parallelism — 128 protein chains per NeuronCore (batch on
partitions), 512 residues on the free axis, position-major permuted layout
(residue r = G*j + i at column i*NB + j; G=8 positions, NB=64 blocks).

All elementwise work runs on DVE in fp16 (2x perf mode: 2-byte dtype +
unit innermost stride); fp16's 11-bit mantissa keeps the scan accurate.
ScalarE carries trig, copies and replication; GpSimd only iota/memsets.

Key throughput trick: the quaternion scan state is PACKED per slab as
[4 comps x NB] so one combine is 11 instructions — the 16 cross-component
products collapse into 4 ops whose right operand uses component-shuffle
access patterns (affine, incl. negative strides), and the add tree is
partially fused across components.  The rotation stage is fused across the
three atom vectors (component-major u/w tiles) with the identity term,
doubling and chain-length mask folded into the combo planes.
"""

import math

import dataclasses

import numpy as np

N_CORES = 8
B_FULL = 1024
L = 512
CB = B_FULL // N_CORES
NB = 64   # scan blocks
G = L // NB  # 8 positions per block
NL = L - NB  # 448
ST = 4 * NB  # state width per slab (4 comps)

R_CA_C = 1.525
R_C_N = 1.330
R_N_CA = 1.460
CA_C_N = math.pi - 2.1186
C_N_CA = math.pi - 1.9391
N_CA_C = math.pi - 2.061

B_K = [C_N_CA, N_CA_C, CA_C_N]
R_KC = [R_C_N, R_N_CA, R_CA_C]

HALF_PI = math.pi / 2.0


def _mkap(base_ap, off, dims):
    """Raw AP from a tile's base [partition, free] AP: free dims replaced by
    `dims` ([stride, count] pairs, element units), offset advanced by `off`."""
    import concourse.mybir as mybir

    part = list(base_ap.ap[0])
    return dataclasses.replace(
        base_ap,
        offset=base_ap.offset + off,
        ap=mybir.VecI64Pair([part] + [list(d) for d in dims]),
    )


def _body(ctx, tc, out_ap, inp_ap, lens_ap):
    import concourse.mybir as mybir

    nc = tc.nc
    f32 = mybir.dt.float32
    h16 = mybir.dt.float16
    Alu = mybir.AluOpType
    Act = mybir.ActivationFunctionType

    cb0h, sb0h = math.cos(B_K[0] / 2), math.sin(B_K[0] / 2)
    cb1h, sb1h = math.cos(B_K[1] / 2), math.sin(B_K[1] / 2)
    cb2h, sb2h = math.cos(B_K[2] / 2), math.sin(B_K[2] / 2)
    cb0f, sb0f = math.cos(B_K[0]), math.sin(B_K[0])
    cb1f, sb1f = math.cos(B_K[1]), math.sin(B_K[1])

    def ttv(o, a, b, op):
        nc.vector.tensor_tensor(out=o, in0=a, in1=b, op=op)

    def stt(o, in0, scalar, in1, op0, op1):
        nc.vector.scalar_tensor_tensor(out=o, in0=in0, scalar=scalar, in1=in1,
                                       op0=op0, op1=op1)

    def ts(o, a, s1, s2=None):
        nc.scalar.activation(o, a, Act.Identity,
                             bias=(0.0 if s2 is None else cval(s2)), scale=s1)

    def ts_v(o, a, s1, s2=None):
        if s2 is None:
            nc.vector.tensor_scalar(out=o, in0=a, scalar1=s1, scalar2=None,
                                    op0=Alu.mult)
        else:
            nc.vector.tensor_scalar(out=o, in0=a, scalar1=s1, scalar2=s2,
                                    op0=Alu.mult, op1=Alu.add)

    def acopy(o, a):
        nc.scalar.activation(o, a, Act.Copy, bias=0.0, scale=1.0)

    # ------------------------------------------------------------------ pools
    persist = ctx.enter_context(tc.tile_pool(name="persist", bufs=1))
    Qs = persist.tile([CB, G * ST], h16, name="Qs")  # packed scan state
    # component-major u/w tiles: [:, k, :] = vector k's component plane
    UX = persist.tile([CB, 3, L], h16, name="UX")
    UY = persist.tile([CB, 3, L], h16, name="UY")
    UZ = persist.tile([CB, 3, L], h16, name="UZ")
    WX = persist.tile([CB, 3, L], h16, name="WX")
    WY = persist.tile([CB, 3, L], h16, name="WY")
    WZ = persist.tile([CB, 3, L], h16, name="WZ")
    cfb = [persist.tile([CB, L], h16, name=f"cfb{i}") for i in range(3)]
    sfb = [persist.tile([CB, L], h16, name=f"sfb{i}") for i in range(3)]
    out_sb = persist.tile([CB, 9 * L], h16, name="out_sb")
    ones = persist.tile([CB, NB], f32, name="ones")
    maskb = persist.tile([CB, L], h16, name="maskb")
    lens_sb = persist.tile([CB, 1], f32, name="lens_sb")
    warm = persist.tile([CB, 2], h16, name="warm")

    nc.gpsimd.memset(ones[:], 1.0)
    # prefetch the ScalarE activation table during the input DMA
    nc.scalar.activation(warm[:], ones[:][:, 0:2], Act.Sin, bias=0.0, scale=1.0)
    nc.sync.dma_start(lens_sb[:], lens_ap)

    _consts = {}

    def cval(v):
        if v not in _consts:
            t = persist.tile([CB, 1], f32, name=f"cval_{len(_consts)}")
            nc.gpsimd.memset(t[:], v)
            _consts[v] = t[:]
        return _consts[v]

    # scan temporaries (packed product planes with accumulator at 16n)
    scan_pool = ctx.enter_context(tc.tile_pool(name="scan", bufs=1))
    Pf = scan_pool.tile([CB, 20 * NB], h16, name="Pf")
    Pb = scan_pool.tile([CB, 20 * NL], h16, name="Pb")

    def addtree(Pt, Ot, Obase, Oc, n, ocols=None, pcols=None):
        P, Ob = Pt[:], Ot[:]
        A = 16 * n  # accumulator base inside the merged tile
        if ocols is None:
            ocols = [[1, n]]
        if pcols is None:
            pcols = [[1, n]]
        # fused add tree (cells: group g at 4gn, cell p at +pn):
        # w=(m0)-(m1+m2+m3) x=(m4+m5+m6)-m7 y=(m8+m10+m11)-m9 z=(m12+m13+m15)-m14
        # level 1: pairs (w&y fused, x&z fused)
        ttv(_mkap(P, A, [[2 * n, 2], [1, n]]),
            _mkap(P, n, [[9 * n, 2], [1, n]]),
            _mkap(P, 2 * n, [[9 * n, 2], [1, n]]), Alu.add)
        ttv(_mkap(P, A + n, [[2 * n, 2], [1, n]]),
            _mkap(P, 4 * n, [[8 * n, 2], [1, n]]),
            _mkap(P, 5 * n, [[8 * n, 2], [1, n]]), Alu.add)
        # level 2: third terms (w&z fused, x&y fused)
        ttv(_mkap(P, A, [[3 * n, 2], [1, n]]),
            _mkap(P, A, [[3 * n, 2], [1, n]]),
            _mkap(P, 3 * n, [[12 * n, 2], [1, n]]), Alu.add)
        ttv(_mkap(P, A + n, [[n, 2], [1, n]]),
            _mkap(P, A + n, [[n, 2], [1, n]]),
            _mkap(P, 6 * n, [[2 * n, 2], [1, n]]), Alu.add)
        # level 3: isolated terms (x&y fused; w&z fused via the merged layout:
        # in0 = [cell(0,0), acc(3)], in1 = [acc(0), cell(3,2)], both affine)
        ttv(_mkap(Ob, Obase + Oc, [[Oc, 2]] + ocols),
            _mkap(P, A + n, [[n, 2]] + pcols),
            _mkap(P, 7 * n, [[2 * n, 2]] + pcols), Alu.subtract)
        ttv(_mkap(Ob, Obase, [[3 * Oc, 2]] + ocols),
            _mkap(P, 0, [[A + 3 * n, 2]] + pcols),
            _mkap(P, A, [[-2 * n, 2]] + pcols), Alu.subtract)

    def qcombine(Lt, Lbase, Lc, Rt, Rbase, Rc, Ot, Obase, Oc, n, Pt,
                 lcols=None, rcols=None, pcols=None):
        """Packed-state quaternion combine: O = L (x) R.
        (tile, base, comp-stride) triples; n = active columns; P temps.
        lcols/rcols/pcols override the column dims (default contiguous)."""
        Lb, Rb, Ob = Lt[:], Rt[:], Ot[:]
        P = Pt[:]
        if lcols is None:
            lcols = [[1, n]]
        if rcols is None:
            rcols = [[1, n]]
        if pcols is None:
            pcols = [[1, n]]

        def nat22(t, b, C):
            return _mkap(t, b, [[2 * C, 2], [C, 2]] + lcols)

        sig = {
            0: lambda t, b, C: _mkap(t, b, [[2 * C, 2], [C, 2]] + rcols),
            1: lambda t, b, C: _mkap(t, b + C, [[2 * C, 2], [-C, 2]] + rcols),
            2: lambda t, b, C: _mkap(t, b + 2 * C, [[-2 * C, 2], [C, 2]] + rcols),
            3: lambda t, b, C: _mkap(t, b + 3 * C, [[-C, 4]] + rcols),
        }
        # products: P[g] (4 cells, L-comp order) = L_nat * R_sigma(g)
        for g in range(4):
            dst = _mkap(P, g * 4 * n, [[2 * n, 2], [n, 2]] + pcols)
            ttv(dst, nat22(Lb, Lbase, Lc), sig[g](Rb, Rbase, Rc), Alu.mult)
        addtree(Pt, Ot, Obase, Oc, n, ocols=rcols, pcols=pcols)

    # ------------------------------------------------------ phase A/B1 planes
    phase_b = tc.tile_pool(name="phase_b", bufs=1)
    pb = phase_b.__enter__()
    dih = pb.tile([CB, 3, L], f32, name="dih")
    # phi+psi first (4KB rows) so ssum/sdif and the S/D trig start before
    # omega lands; omega (2KB rows) second
    nc.sync.dma_start(dih[:][:, 0:2, :], inp_ap[:, 0:2, :])
    nc.sync.dma_start(dih[:][:, 2, :], inp_ap[:, 2, :])

    def bplane(name, dt_=h16):
        return pb.tile([CB, L], dt_, name=name)

    cf = [bplane(f"cf{i}") for i in range(3)]
    sf = [bplane(f"sf{i}") for i in range(3)]
    sOh = bplane("sOh")
    ssum = bplane("ssum", f32)
    sdif = bplane("sdif", f32)
    trig4 = pb.tile([CB, 4 * L], h16, name="trig4")  # cS|sS|cD|sD planes
    cOh = bplane("cOh")
    q3p = pb.tile([CB, 4 * L], h16, name="q3p")
    q4p = pb.tile([CB, 4 * L], h16, name="q4p")
    qtp = pb.tile([CB, 4 * L], h16, name="qtp")
    iota = bplane("iota", f32)

    # mask = (r < length); iota value r = G*j+i at permuted col i*NB+j
    # (early: the masked cfb/sfb planes need it)
    nc.gpsimd.iota(iota[:], pattern=[[1, G], [G, NB]], base=0,
                   channel_multiplier=0, allow_small_or_imprecise_dtypes=True)
    nc.vector.tensor_scalar(out=maskb[:], in0=iota[:], scalar1=lens_sb[:],
                            scalar2=None, op0=Alu.is_lt)

    # B1 in two halves (cols [0:256], [256:512]) so ScalarE trig pipelines
    # with the DVE rotor build; L1 scan steps interleave after.  Order within
    # a half: ssum/sdif (DVE, input-only dep) and the rotor-critical SINs
    # first so DVE never waits on the u-path trig.
    HW_ = L // 2

    for h in range(2):
        lo = h * HW_
        hi_ = lo + HW_

        def S(p):
            return p[:][:, lo:hi_]

        # input is pre-permuted on the host: flat views everywhere
        def ang(k):
            return _mkap(dih[:], k * L + lo, [[1, HW_]])

        phi, psi, omg = ang(0), ang(1), ang(2)

        def T4(c):  # trig plane (cS|sS|cD|sD), current half
            return _mkap(trig4[:], c * L + lo, [[1, HW_]])

        def T4p(c0, step):  # plane pair view
            return _mkap(trig4[:], c0 * L + lo, [[step * L, 2], [1, HW_]])

        ttv(S(ssum), phi, psi, Alu.add)
        ttv(S(sdif), phi, psi, Alu.subtract)
        nc.scalar.activation(T4(1), S(ssum), Act.Sin, bias=0.0, scale=0.5)
        nc.scalar.activation(T4(3), S(sdif), Act.Sin, bias=0.0, scale=0.5)
        nc.scalar.activation(T4(0), S(ssum), Act.Sin, bias=0.0, scale=0.25)
        ttv(T4(0), T4(0), T4(0), Alu.mult)
        ts_v(T4(0), T4(0), -2.0, 1.0)
        nc.scalar.activation(T4(2), S(sdif), Act.Sin, bias=0.0, scale=0.25)
        ttv(T4(2), T4(2), T4(2), Alu.mult)
        ts_v(T4(2), T4(2), -2.0, 1.0)
        nc.scalar.activation(S(sOh), omg, Act.Sin, bias=0.0, scale=0.5)
        nc.scalar.activation(S(cOh), omg, Act.Sin, bias=cval(HALF_PI),
                             scale=0.5)
        # u-path trig on ScalarE (consumed after L1; half-angle sin lands in
        # cf[k] and is squared in place later)
        nc.scalar.activation(S(sf[0]), phi, Act.Sin, bias=0.0, scale=1.0)
        nc.scalar.activation(S(sf[1]), psi, Act.Sin, bias=0.0, scale=1.0)
        nc.scalar.activation(S(sf[2]), omg, Act.Sin, bias=0.0, scale=1.0)
        for k, angv in enumerate((phi, psi, omg)):
            nc.scalar.activation(S(cf[k]), angv, Act.Sin, bias=0.0, scale=0.5)

        # q3 = qz(phi)qx(b0)qz(psi)qx(b1) from S/D trig
        def q3v(c):
            return _mkap(q3p[:], c * L + lo, [[1, HW_]])

        def q4v(c):
            return _mkap(q4p[:], c * L + lo, [[1, HW_]])

        # sign-compatible pairs with shared scalars: (q30,q33) both subtract,
        # (q31,q32) both add; temps staged in qtp planes
        ts_v(_mkap(qtp[:], lo, [[L, 2], [1, HW_]]), T4p(2, 1), sb0h * sb1h)
        ts_v(_mkap(qtp[:], 2 * L + lo, [[L, 2], [1, HW_]]), T4p(2, 1),
             sb0h * cb1h)
        stt(_mkap(q3p[:], lo, [[3 * L, 2], [1, HW_]]), T4p(0, 1),
            cb0h * cb1h, _mkap(qtp[:], lo, [[L, 2], [1, HW_]]),
            Alu.mult, Alu.subtract)
        stt(_mkap(q3p[:], L + lo, [[L, 2], [1, HW_]]), T4p(0, 1),
            cb0h * sb1h, _mkap(qtp[:], 2 * L + lo, [[L, 2], [1, HW_]]),
            Alu.mult, Alu.add)

        # q4 = q3 * qz(omega/2): packed (A = q3*cOh, B = rev(q3)*sOh, A -+ B)
        ttv(_mkap(q4p[:], lo, [[L, 4], [1, HW_]]),
            _mkap(q3p[:], lo, [[L, 4], [1, HW_]]),
            _mkap(cOh[:], lo, [[0, 4], [1, HW_]]), Alu.mult)
        ttv(_mkap(qtp[:], lo, [[L, 4], [1, HW_]]),
            _mkap(q3p[:], 3 * L + lo, [[-L, 4], [1, HW_]]),
            _mkap(sOh[:], lo, [[0, 4], [1, HW_]]), Alu.mult)
        ttv(_mkap(q4p[:], lo, [[2 * L, 2], [1, HW_]]),
            _mkap(q4p[:], lo, [[2 * L, 2], [1, HW_]]),
            _mkap(qtp[:], lo, [[2 * L, 2], [1, HW_]]), Alu.subtract)
        ttv(_mkap(q4p[:], L + lo, [[2 * L, 2], [1, HW_]]),
            _mkap(q4p[:], L + lo, [[2 * L, 2], [1, HW_]]),
            _mkap(qtp[:], L + lo, [[2 * L, 2], [1, HW_]]), Alu.add)

        # Q = q4 * qx(b2h) -> packed state: slab s comp c at s*ST + c*NB.
        # (stt output is limited to 2 free dims, so no comp-pair packing
        # here; swapped scaled q4 comps staged in qtp planes, dead after q4.)
        ts_v(_mkap(qtp[:], lo, [[3 * L, 2], [1, HW_]]),
             _mkap(q4p[:], L + lo, [[L, 2], [1, HW_]]), sb2h)
        ts_v(_mkap(qtp[:], L + lo, [[L, 2], [1, HW_]]),
             _mkap(q4p[:], lo, [[3 * L, 2], [1, HW_]]), sb2h)

        def stview(c):
            return _mkap(Qs[:], 4 * h * ST + c * NB, [[ST, 4], [1, NB]])

        def h4(t, c):
            return _mkap(t[:], c * L + lo, [[NB, 4], [1, NB]])

        stt(stview(0), h4(q4p, 0), cb2h, h4(qtp, 0), Alu.mult, Alu.subtract)
        stt(stview(1), h4(q4p, 1), cb2h, h4(qtp, 1), Alu.mult, Alu.add)
        stt(stview(2), h4(q4p, 2), cb2h, h4(qtp, 2), Alu.mult, Alu.add)
        stt(stview(3), h4(q4p, 3), cb2h, h4(qtp, 3), Alu.mult, Alu.subtract)

        # L1 serial scan steps available after this half
        for i in range(max(1, 4 * h), 4 * h + 4):
            qcombine(Qs, (i - 1) * ST, NB, Qs, i * ST, NB, Qs, i * ST, NB,
                     NB, Pf)

        # u-path planes (masked): cos = 1-2*sin(a/2)^2, then fold the chain
        # mask so every downstream u/w plane is pre-masked (valid columns only
        # ever consume valid residues, so this equals masking final coords)
        for k in range(3):
            ttv(S(cf[k]), S(cf[k]), S(cf[k]), Alu.mult)
            ts_v(S(cf[k]), S(cf[k]), -2.0, 1.0)
            ttv(S(cfb[k]), S(cf[k]), S(maskb), Alu.mult)
            ttv(S(sfb[k]), S(sf[k]), S(maskb), Alu.mult)

    # ---------------- Phase B2: u vectors (fp16; mostly ScalarE+DVE) ----------
    def uview(t, k):
        return t[:][:, k, :]

    p1 = scan_pool.tile([CB, L], h16, name="p1")
    p2 = scan_pool.tile([CB, L], h16, name="p2")
    p3 = scan_pool.tile([CB, L], h16, name="p3")
    p4 = scan_pool.tile([CB, L], h16, name="p4")
    ttv(p1[:], cfb[0][:], cfb[1][:], Alu.mult)
    ttv(p2[:], sfb[0][:], sfb[1][:], Alu.mult)
    ttv(p3[:], sfb[0][:], cfb[1][:], Alu.mult)
    ttv(p4[:], cfb[0][:], sfb[1][:], Alu.mult)

    v0p = scan_pool.tile([CB, 3 * L], h16, name="v0p")
    v0 = [v0p[:][:, d * L:(d + 1) * L] for d in range(3)]
    bt1 = scan_pool.tile([CB, L], h16, name="bt1")
    bt2 = scan_pool.tile([CB, L], h16, name="bt2")
    ts(bt1[:], p2[:], -cb0f)
    ttv(v0[0], bt1[:], p1[:], Alu.add)
    ts(bt2[:], p4[:], cb0f)
    ttv(v0[1], bt2[:], p3[:], Alu.add)
    ts(v0[2], sfb[1][:], sb0f)

    ts(uview(UX, 0), cfb[0][:], R_KC[0])
    ts(uview(UY, 0), sfb[0][:], R_KC[0])
    nc.gpsimd.memset(UZ[:][:, 0, :], 0.0)
    nc.gpsimd.memset(UX[:][:, 0, 0:1], 0.0)
    nc.gpsimd.memset(UY[:][:, 0, 0:1], 0.0)

    ts(bt1[:], v0[0], R_KC[1])
    ttv(uview(UX, 1), bt1[:], uview(UX, 0), Alu.add)
    ts(bt2[:], v0[1], R_KC[1])
    ttv(uview(UY, 1), bt2[:], uview(UY, 0), Alu.add)
    ts(uview(UZ, 1), v0[2][:], R_KC[1])

    c1p = scan_pool.tile([CB, 3 * L], h16, name="c1p")
    c1x = c1p[:][:, 0:L]
    c1y = c1p[:][:, L:2 * L]
    c1z = c1p[:][:, 2 * L:3 * L]
    ts(c1x, sfb[0][:], sb0f * sb1f)
    ts(bt1[:], p3[:], -cb0f * cb1f)
    ttv(c1x, bt1[:], c1x, Alu.add)
    ts(bt2[:], p4[:], -cb1f)
    ttv(c1x, bt2[:], c1x, Alu.add)
    ts(c1y, cfb[0][:], -sb0f * sb1f)
    ts(bt1[:], p1[:], cb0f * cb1f)
    ttv(c1y, bt1[:], c1y, Alu.add)
    ts(bt2[:], p2[:], -cb1f)
    ttv(c1y, bt2[:], c1y, Alu.add)
    ts(c1z, cfb[1][:], sb0f * cb1f, cb0f * sb1f)

    qa = scan_pool.tile([CB, 3 * L], h16, name="u2qa")
    qb = scan_pool.tile([CB, 3 * L], h16, name="u2qb")
    v3 = lambda t: t[:].rearrange("p (a b) -> p a b", b=L)
    ttv(v3(qa), v3(v0p), _mkap(cfb[2][:], 0, [[0, 3], [1, L]]), Alu.mult)
    ttv(v3(qb), v3(c1p), _mkap(sfb[2][:], 0, [[0, 3], [1, L]]), Alu.mult)
    ttv(qa[:], qa[:], qb[:], Alu.add)
    ts(qb[:], qa[:], R_KC[2])
    for d, UT in enumerate((UX, UY, UZ)):
        ttv(uview(UT, 2), qb[:][:, d * L:(d + 1) * L], uview(UT, 1), Alu.add)

    # ---------------- L2: doubling scan over the NB block aggregates ----------
    PQh = scan_pool.tile([CB, 4 * NB], h16, name="PQh")
    Qxb = scan_pool.tile([CB, 4 * L], h16, name="Qxb")
    aggbase = (G - 1) * ST

    # Qxb local-prefix copies depend only on L1 -> issue before L2 so ScalarE
    # runs them under the L2 combines
    for c in range(4):
        lview = _mkap(Qxb[:], c * L + NB, [[NB, G - 1], [1, NB]])
        sview = _mkap(Qs[:], c * NB, [[ST, G - 1], [1, NB]])
        acopy(lview, sview)

    # Hillis doubling scan over the aggregates (DVE APs are limited to 3 free
    # dims — TENSOR3D — so the group-structured Sklansky variant can't lower)
    s = 1
    while s < NB:
        qcombine(Qs, aggbase, NB, Qs, aggbase + s, NB, Qs, aggbase + s, NB,
                 NB - s, Pf)
        s *= 2

    phase_b.__exit__(None, None, None)
    rot_pool = ctx.enter_context(tc.tile_pool(name="rot", bufs=1))

    # ---------------- prefix apply (fp16, packed, broadcast prefix) ----------
    for c in range(4):
        nc.gpsimd.memset(PQh[:][:, c * NB:c * NB + 1], 1.0 if c == 0 else 0.0)
        acopy(PQh[:][:, c * NB + 1:(c + 1) * NB],
              Qs[:][:, aggbase + c * NB:aggbase + c * NB + NB - 1])
        acopy(Qxb[:][:, c * L:c * L + NB], PQh[:][:, c * NB:(c + 1) * NB])

    # products read the un-replicated block prefix with a stride-0 slab dim
    _SIG_PAIRS = {0: ((0, 1), (2, 1)), 1: ((1, -1), (3, -1)),
                  2: ((2, 1), (0, 1)), 3: ((3, -1), (1, -1))}
    for g in range(4):
        for h in range(2):
            base_c, sgn = _SIG_PAIRS[g][h]
            lap = _mkap(PQh[:], 2 * h * NB, [[NB, 2], [0, G - 1], [1, NB]])
            rap = _mkap(Qxb[:], base_c * L + NB,
                        [[sgn * L, 2], [NB, G - 1], [1, NB]])
            dst = _mkap(Pb[:], (4 * g + 2 * h) * NL,
                        [[NL, 2], [NB, G - 1], [1, NB]])
            ttv(dst, lap, rap, Alu.mult)
    addtree(Pb, Qxb, NB, L, NL)

    # ---------------- fused rotation by Qex -----------------------------------
    # products: xx|yy|zz, xy|xz, wx|wy|wz, yz
    Pr = rot_pool.tile([CB, 6 * L], h16, name="Pr")   # xx yy zz xy xz yz
    Wp = rot_pool.tile([CB, 3 * L], h16, name="Wp")   # wx wy wz
    Scm = rot_pool.tile([CB, 3 * L], h16, name="Scm")  # 1-2*S1..S3
    Acm = rot_pool.tile([CB, 3 * L], h16, name="Acm")  # 2*A1..A3
    Dcm = rot_pool.tile([CB, 3 * L], h16, name="Dcm")  # 2*D1..D3

    def qc(c):  # Qxb comp plane
        return Qxb[:][:, c * L:(c + 1) * L]

    def seg3(t, i, m=1):
        return t[:][:, i * L:(i + m) * L]

    ttv(seg3(Pr, 0, 3).rearrange("p (a b) -> p a b", b=L),
        _mkap(Qxb[:], L, [[L, 3], [1, L]]),
        _mkap(Qxb[:], L, [[L, 3], [1, L]]), Alu.mult)
    ttv(seg3(Pr, 3, 2).rearrange("p (a b) -> p a b", b=L),
        _mkap(Qxb[:], L, [[0, 2], [1, L]]),
        _mkap(Qxb[:], 2 * L, [[L, 2], [1, L]]), Alu.mult)
    ttv(seg3(Pr, 5, 1), qc(2), qc(3), Alu.mult)
    ttv(Wp[:].rearrange("p (a b) -> p a b", b=L),
        _mkap(Qxb[:], 0, [[0, 3], [1, L]]),
        _mkap(Qxb[:], L, [[L, 3], [1, L]]), Alu.mult)
    # combos: S1=yy+zz; (S2,S3)=xx+(zz,yy); A/D = (xy,xz,yz) -+ (wz,wy,wx)
    # then fold the rotation's identity term and doubling in:
    #   Scm <- 1-2*S, Acm <- 2*A, Dcm <- 2*D   so   w_x = ux*S1+uy*D1+uz*A2 etc.
    ttv(seg3(Scm, 0, 1), seg3(Pr, 1, 1), seg3(Pr, 2, 1), Alu.add)
    ttv(seg3(Scm, 1, 2).rearrange("p (a b) -> p a b", b=L),
        _mkap(Pr[:], 0, [[0, 2], [1, L]]),
        _mkap(Pr[:], 2 * L, [[-L, 2], [1, L]]), Alu.add)
    ttv(Acm[:].rearrange("p (a b) -> p a b", b=L),
        _mkap(Pr[:], 3 * L, [[L, 3], [1, L]]),
        _mkap(Wp[:], 2 * L, [[-L, 3], [1, L]]), Alu.add)
    ttv(Dcm[:].rearrange("p (a b) -> p a b", b=L),
        _mkap(Pr[:], 3 * L, [[L, 3], [1, L]]),
        _mkap(Wp[:], 2 * L, [[-L, 3], [1, L]]), Alu.subtract)
    ts_v(Scm[:], Scm[:], -2.0, 1.0)
    ts_v(Acm[:], Acm[:], 2.0)
    ts_v(Dcm[:], Dcm[:], 2.0)

    rt1 = rot_pool.tile([CB, 3 * L], h16, name="rt1")
    rt2 = rot_pool.tile([CB, 3 * L], h16, name="rt2")

    def cb3(t, i):  # combo i broadcast over the 3 atom vectors
        return _mkap(t[:], i * L, [[0, 3], [1, L]])

    def full3(t):
        ap = t[:]
        if len(ap.shape) == 3:
            return ap
        return ap.rearrange("p (a b) -> p a b", b=L)

    # w_x = ux*S1' + uy*D1' + uz*A2'
    ttv(full3(rt1), full3(UY), cb3(Dcm, 0), Alu.mult)
    ttv(full3(rt2), full3(UZ), cb3(Acm, 1), Alu.mult)
    ttv(full3(rt1), full3(rt1), full3(rt2), Alu.add)
    ttv(full3(rt2), full3(UX), cb3(Scm, 0), Alu.mult)
    ttv(full3(WX), full3(rt1), full3(rt2), Alu.add)
    # w_y = ux*A1' + uy*S2' + uz*D3'
    ttv(full3(rt1), full3(UX), cb3(Acm, 0), Alu.mult)
    ttv(full3(rt2), full3(UZ), cb3(Dcm, 2), Alu.mult)
    ttv(full3(rt1), full3(rt1), full3(rt2), Alu.add)
    ttv(full3(rt2), full3(UY), cb3(Scm, 1), Alu.mult)
    ttv(full3(WY), full3(rt1), full3(rt2), Alu.add)
    # w_z = ux*D2' + uy*A3' + uz*S3'
    ttv(full3(rt1), full3(UX), cb3(Dcm, 1), Alu.mult)
    ttv(full3(rt2), full3(UY), cb3(Acm, 2), Alu.mult)
    ttv(full3(rt1), full3(rt1), full3(rt2), Alu.add)
    ttv(full3(rt2), full3(UZ), cb3(Scm, 2), Alu.mult)
    ttv(full3(WZ), full3(rt1), full3(rt2), Alu.add)

    # ---------------- Phase E: residue cumsum + output ------------------------
    # (u planes were pre-masked, so all w planes are already masked)
    # One full-length f32 scan per dim: the multi-dim AP walks positions
    # innermost / blocks outer, so the carry crosses block boundaries in
    # exact residue order (r = G*j + i) — replaces the whole two-level
    # dpl3/Ot/Binc pyramid.
    # (the scan op needs flat 2-dim operands, so ScalarE first copies the
    # w residue totals into residue order r = G*j + i)
    wres = [rot_pool.tile([CB, L], h16, name=f"wres_{d}") for d in range(3)]
    Bf = [rot_pool.tile([CB, L], f32, name=f"Bf_{d}") for d in range(3)]
    Bexm = [rot_pool.tile([CB, L], h16, name=f"Bexm_{d}") for d in range(3)]
    for d, WT in enumerate((WX, WY, WZ)):
        acopy(_mkap(wres[d][:], 0, [[G, NB], [1, G]]),
              _mkap(WT[:], 2 * L, [[1, NB], [NB, G]]))
        nc.vector.tensor_tensor_scan(
            out=Bf[d][:],
            data0=_mkap(ones[:], 0, [[0, L]]),
            data1=wres[d][:],
            initial=0.0, op0=Alu.mult, op1=Alu.add,
        )
        # exclusive masked base back in permuted col layout (col = i*NB+j):
        # col (i,j) <- Bf[G*j+i-1] for i>0; col (0,j) <- Bf[G*j-1] for j>0
        ttv(_mkap(Bexm[d][:], NB, [[NB, G - 1], [1, NB]]),
            _mkap(Bf[d][:], 0, [[1, G - 1], [G, NB]]),
            _mkap(maskb[:], NB, [[NB, G - 1], [1, NB]]), Alu.mult)
        ttv(_mkap(Bexm[d][:], 1, [[1, NB - 1]]),
            _mkap(Bf[d][:], G - 1, [[G, NB - 1]]),
            _mkap(maskb[:], 1, [[1, NB - 1]]), Alu.mult)
        nc.vector.memset(Bexm[d][:][:, 0:1], 0.0)

    # natural-form fp16 output: plane-major (q = 3k+d at q*L + permuted
    # col), each k-group's 3 planes DMA'd while the next group's adds run
    wtiles = (WX, WY, WZ)
    for k in range(3):
        for d in range(3):
            q = 3 * k + d
            ttv(out_sb[:][:, q * L:(q + 1) * L], wtiles[d][:][:, k, :],
                Bexm[d][:], Alu.add)
        c0, c1 = 3 * k * L, 3 * (k + 1) * L
        nc.sync.dma_start(out_ap[:, c0:c1], out_sb[:][:, c0:c1])

_CACHE = {}


def _build():
    from contextlib import ExitStack

    import concourse.bacc as bacc
    import concourse.mybir as mybir
    import concourse.tile as tile

    nc = bacc.Bacc("TRN2", target_bir_lowering=False, debug=False,
                   num_devices=N_CORES)
    inp = nc.dram_tensor("input", [CB, 3, L], mybir.dt.float32,
                         kind="ExternalInput").ap()
    lens = nc.dram_tensor("lens", [CB, 1], mybir.dt.float32,
                          kind="ExternalInput").ap()
    out = nc.dram_tensor("out", [CB, 9 * L], mybir.dt.float16,
                         kind="ExternalOutput").ap()
    with tile.TileContext(nc) as tc_ctx, ExitStack() as ctx:
        _body(ctx, tc_ctx, out, inp, lens)
    nc.compile()
    return nc


def get_nc():
    if "nc" not in _CACHE:
        _CACHE["nc"] = _build()
    return _CACHE["nc"]


_PERM = np.arange(L)
_PERM = G * (_PERM % NB) + _PERM // NB  # residue held by permuted col c


def make_in_maps(input, angles_length):
    # stage the angle columns in the kernel's position-major permuted order
    # (col c = i*NB+j holds residue G*j+i) so every B1 view is flat
    inp = np.ascontiguousarray(
        np.asarray(input, dtype=np.float32)[:, :, _PERM])
    lens = np.asarray(angles_length).astype(np.float32).reshape(B_FULL, 1)
    in_maps = []
    for i in range(N_CORES):
        sl = slice(i * CB, (i + 1) * CB)
        in_maps.append({
            "input": np.ascontiguousarray(inp[sl]),
            "lens": np.ascontiguousarray(lens[sl]),
        })
    return in_maps


_COLOF = np.arange(L)
_COLOF = (_COLOF % G) * NB + _COLOF // G  # permuted col holding residue r


def gather_out(outs):
    # device output is fp16 plane-major (q = 3k+d at q*L + permuted col);
    # un-permute to residue order and widen exactly to f32 on the host
    nat = np.concatenate(outs, axis=0).reshape(-1, 9, L)
    return np.ascontiguousarray(
        nat[:, :, _COLOF].transpose(0, 2, 1)).reshape(
        -1, 9 * L).astype(np.float32)


def kernel(input, angles_length):
    from concourse.bass_utils import run_bass_kernel_spmd

    nc = get_nc()
    in_maps = make_in_maps(input, angles_length)
    res = run_bass_kernel_spmd(nc, in_maps, core_ids=list(range(N_CORES)))
    return gather_out([res.results[i]["out"] for i in range(N_CORES)])


# revision 49
# speedup vs baseline: 1.1915x; 1.1915x over previous
"""Trainium2 Bass kernel for nn_Angles2Backbone.

Full inputs:  input [1024, 3, 512] f32 (phi/psi/omega dihedrals), angles_length [1024] i64.
Full output:  [1024, 4608] f32 backbone coords (N, CA, C per residue, xyz interleaved).

Strategy: pure data parallelism — 128 protein chains per NeuronCore (batch on
partitions), 512 residues on the free axis, position-major permuted layout
(residue r = G*j + i at column i*NB + j; G=8 positions, NB=64 blocks).

All elementwise work runs on DVE in fp16 (2x perf mode: 2-byte dtype +
unit innermost stride); fp16's 11-bit mantissa keeps the scan accurate.
ScalarE carries trig, copies and replication; GpSimd only iota/memsets.

Key throughput trick: the quaternion scan state is PACKED per slab as
[4 comps x NB] so one combine is 11 instructions — the 16 cross-component
products collapse into 4 ops whose right operand uses component-shuffle
access patterns (affine, incl. negative strides), and the add tree is
partially fused across components.  The rotation stage is fused across the
three atom vectors (component-major u/w tiles) with the identity term,
doubling and chain-length mask folded into the combo planes.
"""

import math

import dataclasses

import numpy as np

N_CORES = 8
B_FULL = 1024
L = 512
CB = B_FULL // N_CORES
NB = 64   # scan blocks
G = L // NB  # 8 positions per block
NL = L - NB  # 448
ST = 4 * NB  # state width per slab (4 comps)

R_CA_C = 1.525
R_C_N = 1.330
R_N_CA = 1.460
CA_C_N = math.pi - 2.1186
C_N_CA = math.pi - 1.9391
N_CA_C = math.pi - 2.061

B_K = [C_N_CA, N_CA_C, CA_C_N]
R_KC = [R_C_N, R_N_CA, R_CA_C]

HALF_PI = math.pi / 2.0


def _mkap(base_ap, off, dims):
    """Raw AP from a tile's base [partition, free] AP: free dims replaced by
    `dims` ([stride, count] pairs, element units), offset advanced by `off`."""
    import concourse.mybir as mybir

    part = list(base_ap.ap[0])
    return dataclasses.replace(
        base_ap,
        offset=base_ap.offset + off,
        ap=mybir.VecI64Pair([part] + [list(d) for d in dims]),
    )


def _body(ctx, tc, out_ap, inp_ap, lens_ap):
    import concourse.mybir as mybir

    nc = tc.nc
    f32 = mybir.dt.float32
    h16 = mybir.dt.float16
    Alu = mybir.AluOpType
    Act = mybir.ActivationFunctionType

    cb0h, sb0h = math.cos(B_K[0] / 2), math.sin(B_K[0] / 2)
    cb1h, sb1h = math.cos(B_K[1] / 2), math.sin(B_K[1] / 2)
    cb2h, sb2h = math.cos(B_K[2] / 2), math.sin(B_K[2] / 2)
    cb0f, sb0f = math.cos(B_K[0]), math.sin(B_K[0])
    cb1f, sb1f = math.cos(B_K[1]), math.sin(B_K[1])

    def ttv(o, a, b, op):
        nc.vector.tensor_tensor(out=o, in0=a, in1=b, op=op)

    def stt(o, in0, scalar, in1, op0, op1):
        nc.vector.scalar_tensor_tensor(out=o, in0=in0, scalar=scalar, in1=in1,
                                       op0=op0, op1=op1)

    def ts(o, a, s1, s2=None):
        nc.scalar.activation(o, a, Act.Identity,
                             bias=(0.0 if s2 is None else cval(s2)), scale=s1)

    def ts_v(o, a, s1, s2=None):
        if s2 is None:
            nc.vector.tensor_scalar(out=o, in0=a, scalar1=s1, scalar2=None,
                                    op0=Alu.mult)
        else:
            nc.vector.tensor_scalar(out=o, in0=a, scalar1=s1, scalar2=s2,
                                    op0=Alu.mult, op1=Alu.add)

    def acopy(o, a):
        nc.scalar.activation(o, a, Act.Copy, bias=0.0, scale=1.0)

    # ------------------------------------------------------------------ pools
    persist = ctx.enter_context(tc.tile_pool(name="persist", bufs=1))
    Qs = persist.tile([CB, G * ST], h16, name="Qs")  # packed scan state
    # component-major u/w tiles: [:, k, :] = vector k's component plane
    UX = persist.tile([CB, 3, L], h16, name="UX")
    UY = persist.tile([CB, 3, L], h16, name="UY")
    UZ = persist.tile([CB, 3, L], h16, name="UZ")
    WX = persist.tile([CB, 3, L], h16, name="WX")
    WY = persist.tile([CB, 3, L], h16, name="WY")
    WZ = persist.tile([CB, 3, L], h16, name="WZ")
    cfb = [persist.tile([CB, L], h16, name=f"cfb{i}") for i in range(3)]
    sfb = [persist.tile([CB, L], h16, name=f"sfb{i}") for i in range(3)]
    out_sb = persist.tile([CB, 9 * L], h16, name="out_sb")
    ones = persist.tile([CB, NB], f32, name="ones")
    maskb = persist.tile([CB, L], h16, name="maskb")
    lens_sb = persist.tile([CB, 1], f32, name="lens_sb")
    warm = persist.tile([CB, 2], h16, name="warm")

    nc.gpsimd.memset(ones[:], 1.0)
    # prefetch the ScalarE activation table during the input DMA
    nc.scalar.activation(warm[:], ones[:][:, 0:2], Act.Sin, bias=0.0, scale=1.0)
    nc.sync.dma_start(lens_sb[:], lens_ap)

    _consts = {}

    def cval(v):
        if v not in _consts:
            t = persist.tile([CB, 1], f32, name=f"cval_{len(_consts)}")
            nc.gpsimd.memset(t[:], v)
            _consts[v] = t[:]
        return _consts[v]

    # scan temporaries (packed product planes with accumulator at 16n)
    scan_pool = ctx.enter_context(tc.tile_pool(name="scan", bufs=1))
    Pf = scan_pool.tile([CB, 20 * NB], h16, name="Pf")
    Pb = scan_pool.tile([CB, 20 * NL], h16, name="Pb")

    def addtree(Pt, Ot, Obase, Oc, n, ocols=None, pcols=None):
        P, Ob = Pt[:], Ot[:]
        A = 16 * n  # accumulator base inside the merged tile
        if ocols is None:
            ocols = [[1, n]]
        if pcols is None:
            pcols = [[1, n]]
        # fused add tree (cells: group g at 4gn, cell p at +pn):
        # w=(m0)-(m1+m2+m3) x=(m4+m5+m6)-m7 y=(m8+m10+m11)-m9 z=(m12+m13+m15)-m14
        # level 1: pairs (w&y fused, x&z fused)
        ttv(_mkap(P, A, [[2 * n, 2], [1, n]]),
            _mkap(P, n, [[9 * n, 2], [1, n]]),
            _mkap(P, 2 * n, [[9 * n, 2], [1, n]]), Alu.add)
        ttv(_mkap(P, A + n, [[2 * n, 2], [1, n]]),
            _mkap(P, 4 * n, [[8 * n, 2], [1, n]]),
            _mkap(P, 5 * n, [[8 * n, 2], [1, n]]), Alu.add)
        # level 2: third terms (w&z fused, x&y fused)
        ttv(_mkap(P, A, [[3 * n, 2], [1, n]]),
            _mkap(P, A, [[3 * n, 2], [1, n]]),
            _mkap(P, 3 * n, [[12 * n, 2], [1, n]]), Alu.add)
        ttv(_mkap(P, A + n, [[n, 2], [1, n]]),
            _mkap(P, A + n, [[n, 2], [1, n]]),
            _mkap(P, 6 * n, [[2 * n, 2], [1, n]]), Alu.add)
        # level 3: isolated terms (x&y fused; w&z fused via the merged layout:
        # in0 = [cell(0,0), acc(3)], in1 = [acc(0), cell(3,2)], both affine)
        ttv(_mkap(Ob, Obase + Oc, [[Oc, 2]] + ocols),
            _mkap(P, A + n, [[n, 2]] + pcols),
            _mkap(P, 7 * n, [[2 * n, 2]] + pcols), Alu.subtract)
        ttv(_mkap(Ob, Obase, [[3 * Oc, 2]] + ocols),
            _mkap(P, 0, [[A + 3 * n, 2]] + pcols),
            _mkap(P, A, [[-2 * n, 2]] + pcols), Alu.subtract)

    def qcombine(Lt, Lbase, Lc, Rt, Rbase, Rc, Ot, Obase, Oc, n, Pt,
                 lcols=None, rcols=None, pcols=None):
        """Packed-state quaternion combine: O = L (x) R.
        (tile, base, comp-stride) triples; n = active columns; P temps.
        lcols/rcols/pcols override the column dims (default contiguous)."""
        Lb, Rb, Ob = Lt[:], Rt[:], Ot[:]
        P = Pt[:]
        if lcols is None:
            lcols = [[1, n]]
        if rcols is None:
            rcols = [[1, n]]
        if pcols is None:
            pcols = [[1, n]]

        def nat22(t, b, C):
            return _mkap(t, b, [[2 * C, 2], [C, 2]] + lcols)

        sig = {
            0: lambda t, b, C: _mkap(t, b, [[2 * C, 2], [C, 2]] + rcols),
            1: lambda t, b, C: _mkap(t, b + C, [[2 * C, 2], [-C, 2]] + rcols),
            2: lambda t, b, C: _mkap(t, b + 2 * C, [[-2 * C, 2], [C, 2]] + rcols),
            3: lambda t, b, C: _mkap(t, b + 3 * C, [[-C, 4]] + rcols),
        }
        # products: P[g] (4 cells, L-comp order) = L_nat * R_sigma(g)
        for g in range(4):
            dst = _mkap(P, g * 4 * n, [[2 * n, 2], [n, 2]] + pcols)
            ttv(dst, nat22(Lb, Lbase, Lc), sig[g](Rb, Rbase, Rc), Alu.mult)
        addtree(Pt, Ot, Obase, Oc, n, ocols=rcols, pcols=pcols)

    # ------------------------------------------------------ phase A/B1 planes
    phase_b = tc.tile_pool(name="phase_b", bufs=1)
    pb = phase_b.__enter__()
    dih = pb.tile([CB, 3, L], f32, name="dih")
    # phi+psi first (4KB rows) so ssum/sdif and the S/D trig start before
    # omega lands; omega (2KB rows) second
    nc.sync.dma_start(dih[:][:, 0:2, :], inp_ap[:, 0:2, :])
    nc.sync.dma_start(dih[:][:, 2, :], inp_ap[:, 2, :])

    def bplane(name, dt_=h16):
        return pb.tile([CB, L], dt_, name=name)

    cf = [bplane(f"cf{i}") for i in range(3)]
    sf = [bplane(f"sf{i}") for i in range(3)]
    sOh = bplane("sOh")
    ssum = bplane("ssum", f32)
    sdif = bplane("sdif", f32)
    trig4 = pb.tile([CB, 4 * L], h16, name="trig4")  # cS|sS|cD|sD planes
    cOh = bplane("cOh")
    q3p = pb.tile([CB, 4 * L], h16, name="q3p")
    q4p = pb.tile([CB, 4 * L], h16, name="q4p")
    qtp = pb.tile([CB, 4 * L], h16, name="qtp")
    iota = bplane("iota", f32)

    # mask = (r < length); iota value r = G*j+i at permuted col i*NB+j
    # (early: the masked cfb/sfb planes need it)
    nc.gpsimd.iota(iota[:], pattern=[[1, G], [G, NB]], base=0,
                   channel_multiplier=0, allow_small_or_imprecise_dtypes=True)
    nc.vector.tensor_scalar(out=maskb[:], in0=iota[:], scalar1=lens_sb[:],
                            scalar2=None, op0=Alu.is_lt)

    # B1 in two halves (cols [0:256], [256:512]) so ScalarE trig pipelines
    # with the DVE rotor build; L1 scan steps interleave after.  Order within
    # a half: ssum/sdif (DVE, input-only dep) and the rotor-critical SINs
    # first so DVE never waits on the u-path trig.
    HW_ = L // 2

    for h in range(2):
        lo = h * HW_
        hi_ = lo + HW_

        def S(p):
            return p[:][:, lo:hi_]

        # input is pre-permuted on the host: flat views everywhere
        def ang(k):
            return _mkap(dih[:], k * L + lo, [[1, HW_]])

        phi, psi, omg = ang(0), ang(1), ang(2)

        def T4(c):  # trig plane (cS|sS|cD|sD), current half
            return _mkap(trig4[:], c * L + lo, [[1, HW_]])

        def T4p(c0, step):  # plane pair view
            return _mkap(trig4[:], c0 * L + lo, [[step * L, 2], [1, HW_]])

        ttv(S(ssum), phi, psi, Alu.add)
        ttv(S(sdif), phi, psi, Alu.subtract)
        nc.scalar.activation(T4(1), S(ssum), Act.Sin, bias=0.0, scale=0.5)
        nc.scalar.activation(T4(3), S(sdif), Act.Sin, bias=0.0, scale=0.5)
        nc.scalar.activation(T4(0), S(ssum), Act.Sin, bias=0.0, scale=0.25)
        ttv(T4(0), T4(0), T4(0), Alu.mult)
        ts_v(T4(0), T4(0), -2.0, 1.0)
        nc.scalar.activation(T4(2), S(sdif), Act.Sin, bias=0.0, scale=0.25)
        ttv(T4(2), T4(2), T4(2), Alu.mult)
        ts_v(T4(2), T4(2), -2.0, 1.0)
        nc.scalar.activation(S(sOh), omg, Act.Sin, bias=0.0, scale=0.5)
        nc.scalar.activation(S(cOh), omg, Act.Sin, bias=cval(HALF_PI),
                             scale=0.5)
        # u-path trig on ScalarE (consumed after L1; half-angle sin lands in
        # cf[k] and is squared in place later)
        nc.scalar.activation(S(sf[0]), phi, Act.Sin, bias=0.0, scale=1.0)
        nc.scalar.activation(S(sf[1]), psi, Act.Sin, bias=0.0, scale=1.0)
        nc.scalar.activation(S(sf[2]), omg, Act.Sin, bias=0.0, scale=1.0)
        for k, angv in enumerate((phi, psi, omg)):
            nc.scalar.activation(S(cf[k]), angv, Act.Sin, bias=0.0, scale=0.5)

        # q3 = qz(phi)qx(b0)qz(psi)qx(b1) from S/D trig
        def q3v(c):
            return _mkap(q3p[:], c * L + lo, [[1, HW_]])

        def q4v(c):
            return _mkap(q4p[:], c * L + lo, [[1, HW_]])

        # sign-compatible pairs with shared scalars: (q30,q33) both subtract,
        # (q31,q32) both add; temps staged in qtp planes
        ts_v(_mkap(qtp[:], lo, [[L, 2], [1, HW_]]), T4p(2, 1), sb0h * sb1h)
        ts_v(_mkap(qtp[:], 2 * L + lo, [[L, 2], [1, HW_]]), T4p(2, 1),
             sb0h * cb1h)
        stt(_mkap(q3p[:], lo, [[3 * L, 2], [1, HW_]]), T4p(0, 1),
            cb0h * cb1h, _mkap(qtp[:], lo, [[L, 2], [1, HW_]]),
            Alu.mult, Alu.subtract)
        stt(_mkap(q3p[:], L + lo, [[L, 2], [1, HW_]]), T4p(0, 1),
            cb0h * sb1h, _mkap(qtp[:], 2 * L + lo, [[L, 2], [1, HW_]]),
            Alu.mult, Alu.add)

        # q4 = q3 * qz(omega/2): packed (A = q3*cOh, B = rev(q3)*sOh, A -+ B)
        ttv(_mkap(q4p[:], lo, [[L, 4], [1, HW_]]),
            _mkap(q3p[:], lo, [[L, 4], [1, HW_]]),
            _mkap(cOh[:], lo, [[0, 4], [1, HW_]]), Alu.mult)
        ttv(_mkap(qtp[:], lo, [[L, 4], [1, HW_]]),
            _mkap(q3p[:], 3 * L + lo, [[-L, 4], [1, HW_]]),
            _mkap(sOh[:], lo, [[0, 4], [1, HW_]]), Alu.mult)
        ttv(_mkap(q4p[:], lo, [[2 * L, 2], [1, HW_]]),
            _mkap(q4p[:], lo, [[2 * L, 2], [1, HW_]]),
            _mkap(qtp[:], lo, [[2 * L, 2], [1, HW_]]), Alu.subtract)
        ttv(_mkap(q4p[:], L + lo, [[2 * L, 2], [1, HW_]]),
            _mkap(q4p[:], L + lo, [[2 * L, 2], [1, HW_]]),
            _mkap(qtp[:], L + lo, [[2 * L, 2], [1, HW_]]), Alu.add)

        # Q = q4 * qx(b2h) -> packed state: slab s comp c at s*ST + c*NB.
        # (stt output is limited to 2 free dims, so no comp-pair packing
        # here; swapped scaled q4 comps staged in qtp planes, dead after q4.)
        ts_v(_mkap(qtp[:], lo, [[3 * L, 2], [1, HW_]]),
             _mkap(q4p[:], L + lo, [[L, 2], [1, HW_]]), sb2h)
        ts_v(_mkap(qtp[:], L + lo, [[L, 2], [1, HW_]]),
             _mkap(q4p[:], lo, [[3 * L, 2], [1, HW_]]), sb2h)

        def stview(c):
            return _mkap(Qs[:], 4 * h * ST + c * NB, [[ST, 4], [1, NB]])

        def h4(t, c):
            return _mkap(t[:], c * L + lo, [[NB, 4], [1, NB]])

        stt(stview(0), h4(q4p, 0), cb2h, h4(qtp, 0), Alu.mult, Alu.subtract)
        stt(stview(1), h4(q4p, 1), cb2h, h4(qtp, 1), Alu.mult, Alu.add)
        stt(stview(2), h4(q4p, 2), cb2h, h4(qtp, 2), Alu.mult, Alu.add)
        stt(stview(3), h4(q4p, 3), cb2h, h4(qtp, 3), Alu.mult, Alu.subtract)

        # L1 serial scan steps available after this half
        for i in range(max(1, 4 * h), 4 * h + 4):
            qcombine(Qs, (i - 1) * ST, NB, Qs, i * ST, NB, Qs, i * ST, NB,
                     NB, Pf)

        # u-path planes (masked): cos = 1-2*sin(a/2)^2, then fold the chain
        # mask so every downstream u/w plane is pre-masked (valid columns only
        # ever consume valid residues, so this equals masking final coords)
        for k in range(3):
            ttv(S(cf[k]), S(cf[k]), S(cf[k]), Alu.mult)
            ts_v(S(cf[k]), S(cf[k]), -2.0, 1.0)
            ttv(S(cfb[k]), S(cf[k]), S(maskb), Alu.mult)
            ttv(S(sfb[k]), S(sf[k]), S(maskb), Alu.mult)

    # ---------------- Phase B2: u vectors (fp16; mostly ScalarE+DVE) ----------
    def uview(t, k):
        return t[:][:, k, :]

    p1 = scan_pool.tile([CB, L], h16, name="p1")
    p2 = scan_pool.tile([CB, L], h16, name="p2")
    p3 = scan_pool.tile([CB, L], h16, name="p3")
    p4 = scan_pool.tile([CB, L], h16, name="p4")
    ttv(p1[:], cfb[0][:], cfb[1][:], Alu.mult)
    ttv(p2[:], sfb[0][:], sfb[1][:], Alu.mult)
    ttv(p3[:], sfb[0][:], cfb[1][:], Alu.mult)
    ttv(p4[:], cfb[0][:], sfb[1][:], Alu.mult)

    v0p = scan_pool.tile([CB, 3 * L], h16, name="v0p")
    v0 = [v0p[:][:, d * L:(d + 1) * L] for d in range(3)]
    bt1 = scan_pool.tile([CB, L], h16, name="bt1")
    bt2 = scan_pool.tile([CB, L], h16, name="bt2")
    ts(bt1[:], p2[:], -cb0f)
    ttv(v0[0], bt1[:], p1[:], Alu.add)
    ts(bt2[:], p4[:], cb0f)
    ttv(v0[1], bt2[:], p3[:], Alu.add)
    ts(v0[2], sfb[1][:], sb0f)

    ts(uview(UX, 0), cfb[0][:], R_KC[0])
    ts(uview(UY, 0), sfb[0][:], R_KC[0])
    nc.gpsimd.memset(UZ[:][:, 0, :], 0.0)
    nc.gpsimd.memset(UX[:][:, 0, 0:1], 0.0)
    nc.gpsimd.memset(UY[:][:, 0, 0:1], 0.0)

    ts(bt1[:], v0[0], R_KC[1])
    ttv(uview(UX, 1), bt1[:], uview(UX, 0), Alu.add)
    ts(bt2[:], v0[1], R_KC[1])
    ttv(uview(UY, 1), bt2[:], uview(UY, 0), Alu.add)
    ts(uview(UZ, 1), v0[2][:], R_KC[1])

    c1p = scan_pool.tile([CB, 3 * L], h16, name="c1p")
    c1x = c1p[:][:, 0:L]
    c1y = c1p[:][:, L:2 * L]
    c1z = c1p[:][:, 2 * L:3 * L]
    ts(c1x, sfb[0][:], sb0f * sb1f)
    ts(bt1[:], p3[:], -cb0f * cb1f)
    ttv(c1x, bt1[:], c1x, Alu.add)
    ts(bt2[:], p4[:], -cb1f)
    ttv(c1x, bt2[:], c1x, Alu.add)
    ts(c1y, cfb[0][:], -sb0f * sb1f)
    ts(bt1[:], p1[:], cb0f * cb1f)
    ttv(c1y, bt1[:], c1y, Alu.add)
    ts(bt2[:], p2[:], -cb1f)
    ttv(c1y, bt2[:], c1y, Alu.add)
    ts(c1z, cfb[1][:], sb0f * cb1f, cb0f * sb1f)

    qa = scan_pool.tile([CB, 3 * L], h16, name="u2qa")
    qb = scan_pool.tile([CB, 3 * L], h16, name="u2qb")
    v3 = lambda t: t[:].rearrange("p (a b) -> p a b", b=L)
    ttv(v3(qa), v3(v0p), _mkap(cfb[2][:], 0, [[0, 3], [1, L]]), Alu.mult)
    ttv(v3(qb), v3(c1p), _mkap(sfb[2][:], 0, [[0, 3], [1, L]]), Alu.mult)
    ttv(qa[:], qa[:], qb[:], Alu.add)
    ts(qb[:], qa[:], R_KC[2])
    for d, UT in enumerate((UX, UY, UZ)):
        ttv(uview(UT, 2), qb[:][:, d * L:(d + 1) * L], uview(UT, 1), Alu.add)

    # ---------------- L2: doubling scan over the NB block aggregates ----------
    PQh = scan_pool.tile([CB, 4 * NB], h16, name="PQh")
    Qxb = scan_pool.tile([CB, 4 * L], h16, name="Qxb")
    aggbase = (G - 1) * ST

    # Qxb local-prefix copies depend only on L1 -> issue before L2 so ScalarE
    # runs them under the L2 combines
    for c in range(4):
        lview = _mkap(Qxb[:], c * L + NB, [[NB, G - 1], [1, NB]])
        sview = _mkap(Qs[:], c * NB, [[ST, G - 1], [1, NB]])
        acopy(lview, sview)

    # Hillis doubling scan over the aggregates (DVE APs are limited to 3 free
    # dims — TENSOR3D — so the group-structured Sklansky variant can't lower)
    s = 1
    while s < NB:
        qcombine(Qs, aggbase, NB, Qs, aggbase + s, NB, Qs, aggbase + s, NB,
                 NB - s, Pf)
        s *= 2

    phase_b.__exit__(None, None, None)
    rot_pool = ctx.enter_context(tc.tile_pool(name="rot", bufs=1))

    # ---------------- prefix apply (fp16, packed, broadcast prefix) ----------
    for c in range(4):
        nc.gpsimd.memset(PQh[:][:, c * NB:c * NB + 1], 1.0 if c == 0 else 0.0)
        acopy(PQh[:][:, c * NB + 1:(c + 1) * NB],
              Qs[:][:, aggbase + c * NB:aggbase + c * NB + NB - 1])
        acopy(Qxb[:][:, c * L:c * L + NB], PQh[:][:, c * NB:(c + 1) * NB])

    # products read the un-replicated block prefix with a stride-0 slab dim
    _SIG_PAIRS = {0: ((0, 1), (2, 1)), 1: ((1, -1), (3, -1)),
                  2: ((2, 1), (0, 1)), 3: ((3, -1), (1, -1))}
    for g in range(4):
        for h in range(2):
            base_c, sgn = _SIG_PAIRS[g][h]
            lap = _mkap(PQh[:], 2 * h * NB, [[NB, 2], [0, G - 1], [1, NB]])
            rap = _mkap(Qxb[:], base_c * L + NB,
                        [[sgn * L, 2], [NB, G - 1], [1, NB]])
            dst = _mkap(Pb[:], (4 * g + 2 * h) * NL,
                        [[NL, 2], [NB, G - 1], [1, NB]])
            ttv(dst, lap, rap, Alu.mult)
    addtree(Pb, Qxb, NB, L, NL)

    # ---------------- fused rotation by Qex -----------------------------------
    # products: xx|yy|zz, xy|xz, wx|wy|wz, yz
    Pr = rot_pool.tile([CB, 6 * L], h16, name="Pr")   # xx yy zz xy xz yz
    Wp = rot_pool.tile([CB, 3 * L], h16, name="Wp")   # wx wy wz
    Scm = rot_pool.tile([CB, 3 * L], h16, name="Scm")  # 1-2*S1..S3
    Acm = rot_pool.tile([CB, 3 * L], h16, name="Acm")  # 2*A1..A3
    Dcm = rot_pool.tile([CB, 3 * L], h16, name="Dcm")  # 2*D1..D3

    def qc(c):  # Qxb comp plane
        return Qxb[:][:, c * L:(c + 1) * L]

    def seg3(t, i, m=1):
        return t[:][:, i * L:(i + m) * L]

    ttv(seg3(Pr, 0, 3).rearrange("p (a b) -> p a b", b=L),
        _mkap(Qxb[:], L, [[L, 3], [1, L]]),
        _mkap(Qxb[:], L, [[L, 3], [1, L]]), Alu.mult)
    ttv(seg3(Pr, 3, 2).rearrange("p (a b) -> p a b", b=L),
        _mkap(Qxb[:], L, [[0, 2], [1, L]]),
        _mkap(Qxb[:], 2 * L, [[L, 2], [1, L]]), Alu.mult)
    ttv(seg3(Pr, 5, 1), qc(2), qc(3), Alu.mult)
    ttv(Wp[:].rearrange("p (a b) -> p a b", b=L),
        _mkap(Qxb[:], 0, [[0, 3], [1, L]]),
        _mkap(Qxb[:], L, [[L, 3], [1, L]]), Alu.mult)
    # combos: S1=yy+zz; (S2,S3)=xx+(zz,yy); A/D = (xy,xz,yz) -+ (wz,wy,wx)
    # then fold the rotation's identity term and doubling in:
    #   Scm <- 1-2*S, Acm <- 2*A, Dcm <- 2*D   so   w_x = ux*S1+uy*D1+uz*A2 etc.
    ttv(seg3(Scm, 0, 1), seg3(Pr, 1, 1), seg3(Pr, 2, 1), Alu.add)
    ttv(seg3(Scm, 1, 2).rearrange("p (a b) -> p a b", b=L),
        _mkap(Pr[:], 0, [[0, 2], [1, L]]),
        _mkap(Pr[:], 2 * L, [[-L, 2], [1, L]]), Alu.add)
    ttv(Acm[:].rearrange("p (a b) -> p a b", b=L),
        _mkap(Pr[:], 3 * L, [[L, 3], [1, L]]),
        _mkap(Wp[:], 2 * L, [[-L, 3], [1, L]]), Alu.add)
    ttv(Dcm[:].rearrange("p (a b) -> p a b", b=L),
        _mkap(Pr[:], 3 * L, [[L, 3], [1, L]]),
        _mkap(Wp[:], 2 * L, [[-L, 3], [1, L]]), Alu.subtract)
    ts_v(Scm[:], Scm[:], -2.0, 1.0)
    ts_v(Acm[:], Acm[:], 2.0)
    ts_v(Dcm[:], Dcm[:], 2.0)

    rt1 = rot_pool.tile([CB, 3 * L], h16, name="rt1")
    rt2 = rot_pool.tile([CB, 3 * L], h16, name="rt2")

    def cb3(t, i):  # combo i broadcast over the 3 atom vectors
        return _mkap(t[:], i * L, [[0, 3], [1, L]])

    def full3(t):
        ap = t[:]
        if len(ap.shape) == 3:
            return ap
        return ap.rearrange("p (a b) -> p a b", b=L)

    # w_x = ux*S1' + uy*D1' + uz*A2'
    ttv(full3(rt1), full3(UY), cb3(Dcm, 0), Alu.mult)
    ttv(full3(rt2), full3(UZ), cb3(Acm, 1), Alu.mult)
    ttv(full3(rt1), full3(rt1), full3(rt2), Alu.add)
    ttv(full3(rt2), full3(UX), cb3(Scm, 0), Alu.mult)
    ttv(full3(WX), full3(rt1), full3(rt2), Alu.add)
    # w_y = ux*A1' + uy*S2' + uz*D3'
    ttv(full3(rt1), full3(UX), cb3(Acm, 0), Alu.mult)
    ttv(full3(rt2), full3(UZ), cb3(Dcm, 2), Alu.mult)
    ttv(full3(rt1), full3(rt1), full3(rt2), Alu.add)
    ttv(full3(rt2), full3(UY), cb3(Scm, 1), Alu.mult)
    ttv(full3(WY), full3(rt1), full3(rt2), Alu.add)
    # w_z = ux*D2' + uy*A3' + uz*S3'
    ttv(full3(rt1), full3(UX), cb3(Dcm, 1), Alu.mult)
    ttv(full3(rt2), full3(UY), cb3(Acm, 2), Alu.mult)
    ttv(full3(rt1), full3(rt1), full3(rt2), Alu.add)
    ttv(full3(rt2), full3(UZ), cb3(Scm, 2), Alu.mult)
    ttv(full3(WZ), full3(rt1), full3(rt2), Alu.add)

    # ---------------- Phase E: residue cumsum + output ------------------------
    # (u planes were pre-masked, so all w planes are already masked)
    # One full-length f32 scan per dim: the multi-dim AP walks positions
    # innermost / blocks outer, so the carry crosses block boundaries in
    # exact residue order (r = G*j + i) — replaces the whole two-level
    # dpl3/Ot/Binc pyramid.
    # (the scan op needs flat 2-dim operands, so ScalarE first copies the
    # w residue totals into residue order r = G*j + i)
    wres = [rot_pool.tile([CB, L], h16, name=f"wres_{d}") for d in range(3)]
    Bf = [rot_pool.tile([CB, L], f32, name=f"Bf_{d}") for d in range(3)]
    Bexm = [rot_pool.tile([CB, L], h16, name=f"Bexm_{d}") for d in range(3)]
    for d, WT in enumerate((WX, WY, WZ)):
        acopy(_mkap(wres[d][:], 0, [[G, NB], [1, G]]),
              _mkap(WT[:], 2 * L, [[1, NB], [NB, G]]))
        nc.vector.tensor_tensor_scan(
            out=Bf[d][:],
            data0=_mkap(ones[:], 0, [[0, L]]),
            data1=wres[d][:],
            initial=0.0, op0=Alu.mult, op1=Alu.add,
        )
        # exclusive masked base back in permuted col layout (col = i*NB+j):
        # col (i,j) <- Bf[G*j+i-1] for i>0; col (0,j) <- Bf[G*j-1] for j>0
        ttv(_mkap(Bexm[d][:], NB, [[NB, G - 1], [1, NB]]),
            _mkap(Bf[d][:], 0, [[1, G - 1], [G, NB]]),
            _mkap(maskb[:], NB, [[NB, G - 1], [1, NB]]), Alu.mult)
        ttv(_mkap(Bexm[d][:], 1, [[1, NB - 1]]),
            _mkap(Bf[d][:], G - 1, [[G, NB - 1]]),
            _mkap(maskb[:], 1, [[1, NB - 1]]), Alu.mult)
        nc.vector.memset(Bexm[d][:][:, 0:1], 0.0)

    # natural-form fp16 output, d-outer: plane q=3k+d depends only on
    # Bexm[d], so each dim's planes compute+DMA right after its scan
    wtiles = (WX, WY, WZ)
    for d in range(3):
        for k in range(3):
            q = 3 * k + d
            ttv(out_sb[:][:, q * L:(q + 1) * L], wtiles[d][:][:, k, :],
                Bexm[d][:], Alu.add)
            c0 = q * L
            nc.sync.dma_start(out_ap[:, c0:c0 + L], out_sb[:][:, c0:c0 + L])

_CACHE = {}


def _build():
    from contextlib import ExitStack

    import concourse.bacc as bacc
    import concourse.mybir as mybir
    import concourse.tile as tile

    nc = bacc.Bacc("TRN2", target_bir_lowering=False, debug=False,
                   num_devices=N_CORES)
    inp = nc.dram_tensor("input", [CB, 3, L], mybir.dt.float32,
                         kind="ExternalInput").ap()
    lens = nc.dram_tensor("lens", [CB, 1], mybir.dt.float32,
                          kind="ExternalInput").ap()
    out = nc.dram_tensor("out", [CB, 9 * L], mybir.dt.float16,
                         kind="ExternalOutput").ap()
    with tile.TileContext(nc) as tc_ctx, ExitStack() as ctx:
        _body(ctx, tc_ctx, out, inp, lens)
    nc.compile()
    return nc


def get_nc():
    if "nc" not in _CACHE:
        _CACHE["nc"] = _build()
    return _CACHE["nc"]


_PERM = np.arange(L)
_PERM = G * (_PERM % NB) + _PERM // NB  # residue held by permuted col c


def make_in_maps(input, angles_length):
    # stage the angle columns in the kernel's position-major permuted order
    # (col c = i*NB+j holds residue G*j+i) so every B1 view is flat
    inp = np.ascontiguousarray(
        np.asarray(input, dtype=np.float32)[:, :, _PERM])
    lens = np.asarray(angles_length).astype(np.float32).reshape(B_FULL, 1)
    in_maps = []
    for i in range(N_CORES):
        sl = slice(i * CB, (i + 1) * CB)
        in_maps.append({
            "input": np.ascontiguousarray(inp[sl]),
            "lens": np.ascontiguousarray(lens[sl]),
        })
    return in_maps


_COLOF = np.arange(L)
_COLOF = (_COLOF % G) * NB + _COLOF // G  # permuted col holding residue r


def gather_out(outs):
    # device output is fp16 plane-major (q = 3k+d at q*L + permuted col);
    # un-permute to residue order and widen exactly to f32 on the host
    nat = np.concatenate(outs, axis=0).reshape(-1, 9, L)
    return np.ascontiguousarray(
        nat[:, :, _COLOF].transpose(0, 2, 1)).reshape(
        -1, 9 * L).astype(np.float32)


def kernel(input, angles_length):
    from concourse.bass_utils import run_bass_kernel_spmd

    nc = get_nc()
    in_maps = make_in_maps(input, angles_length)
    res = run_bass_kernel_spmd(nc, in_maps, core_ids=list(range(N_CORES)))
    return gather_out([res.results[i]["out"] for i in range(N_CORES)])
